# revision 10
# baseline (speedup 1.0000x reference)
"""Fused multi-head attention forward (B=2, S=2048, SIZE=1024, H=16) on 8
Trainium2 NeuronCores.

Sharding: 2-way data parallel over batch x 4-way tensor parallel over heads
(Megatron style). Each core computes 4 heads of one batch element end-to-end
(QKV projections for its 256-dim slice, attention, and a partial output
projection); the host sums the 4 partials per batch and adds the output
bias. The value-projection bias drops out of attention algebraically
(softmax rows sum to 1), so the host folds `bv @ Wo.T` into that same
constant row.

On-device layouts avoid all transposes (fp16 operands, fp32 PSUM):
  qhT/khT [dim, s]  <- host-transposed qT/kT as matmul rhs, WqT/WkT as lhsT
  vh65    [s%128, st, head, 65] <- V head tiles with a trailing ONES column:
                       the ctx matmul's output row 64 then accumulates the
                       softmax denominator for free
  scoresT [k, q]    <- khT as lhsT, qhT as rhs; head pairs stream through
                       the PE concurrently on disjoint row groups
  ctxT    [65, q]   <- vh65 as lhsT, exp(scoresT) as rhs
  out     [s, o]    <- ctxT as lhsT, WoT as rhs (c = 128, heads paired)

Schedule: one software pipeline paced by the Activation engine (exp of
S*S*H_loc elements at 1/lane/cycle is the largest single-engine load).
K projection runs up front while vT/weight DMAs stream on the second
DMA-issue queue (Activation's); attention sweeps over (q-chunk, head-pair)
follow. Within a sweep the ctx matmuls are issued TWO kt slots behind the
exp that feeds them so their PE issue is never gated on an Act semaphore;
the scores PSUM pool is 3 deep for the same reason. The V projection is
interleaved into the first sweep's slots; each q-chunk's output projection
is spread in 2-matmul chunks across the next sweep's slots. Softmax
denominators take one DRAM spread/broadcast roundtrip (64-partition
reciprocal); odd heads' normalized ctx takes an SBUF->SBUF DMA to shift to
partitions 64-127 for the paired output projection.
"""

import numpy as np

import concourse.bass as bass
import concourse.tile as tile
from concourse import bacc, mybir
from concourse.bass_utils import run_bass_kernel_spmd

B, S, SIZE, H, D = 2, 2048, 1024, 16, 64
NCORES = 8
HGROUPS = 4                # tensor-parallel head groups
H_LOC = H // HGROUPS       # 4 heads per core
D_LOC = H_LOC * D          # 256 projection dims per core
MT = D_LOC // 128          # 2 head-pairs per core
ET = SIZE // 128           # 8 contraction tiles for projections
ST = S // 128              # 16 sequence tiles of 128
QC = S // 512              # 4 q-chunks of 512
KT = S // 128              # 16 key tiles
LAG = 2                    # ctx matmuls trail exp by this many kt slots
DVE_KTS_LIGHT = (3, 5, 8, 11, 14)  # DVE exp slots in extras-free sweeps
DVE_KTS_HEAVY = (5, 11)            # DVE exp slots in out-proj-carrying sweeps
SCH_A = float(2 ** 10 / np.log(2.0))   # fp16 Schraudolph exp: uint16(A*x+B)
SCH_B = 15315.25                       # bitcast to fp16 ~= e^x (max rel 3.0%)

_NC = None


def build():
    global _NC
    if _NC is not None:
        return _NC
    f32, f16 = mybir.dt.float32, mybir.dt.float16
    u16 = mybir.dt.uint16
    Exp = mybir.ActivationFunctionType.Exp
    Alu = mybir.AluOpType

    nc = bacc.Bacc("TRN2", target_bir_lowering=False, debug=False)
    qT_d = nc.dram_tensor("qT", [SIZE, S], f16, kind="ExternalInput").ap()
    kT_d = nc.dram_tensor("kT", [SIZE, S], f16, kind="ExternalInput").ap()
    vT_d = nc.dram_tensor("vT", [SIZE, S], f16, kind="ExternalInput").ap()
    WqT_d = nc.dram_tensor("WqT", [SIZE, D_LOC], f16, kind="ExternalInput").ap()
    WkT_d = nc.dram_tensor("WkT", [SIZE, D_LOC], f16, kind="ExternalInput").ap()
    WvT_d = nc.dram_tensor("WvT", [SIZE, D_LOC], f16, kind="ExternalInput").ap()
    WoT_d = nc.dram_tensor("WoT", [D_LOC, SIZE], f16, kind="ExternalInput").ap()
    bq_d = nc.dram_tensor("bq", [D_LOC], f32, kind="ExternalInput").ap()
    bk_d = nc.dram_tensor("bk", [D_LOC], f32, kind="ExternalInput").ap()
    out_d = nc.dram_tensor("out", [S, SIZE], f32, kind="ExternalOutput").ap()

    qTt = qT_d.rearrange("(et p) s -> p et s", p=128)
    kTt = kT_d.rearrange("(et p) s -> p et s", p=128)
    vTt = vT_d.rearrange("(et p) s -> p et s", p=128)

    with tile.TileContext(nc) as tc:
        with tc.tile_pool(name="persist", bufs=1) as persist:
            wk_sb = persist.tile([128, ET, D_LOC], f16)
            bk_sb = persist.tile([128, MT], f32)
            WkTt = WkT_d.rearrange("(et p) m -> p et m", p=128)
            for et in range(ET):   # per-et chunks: K matmuls start early
                nc.sync.dma_start(wk_sb[:, et, :], WkTt[:, et, :])
            nc.scalar.dma_start(bk_sb[:], bk_d.rearrange("(mt p) -> p mt", p=128))
            wq_sb = persist.tile([128, ET, D_LOC], f16)
            wv_sb = persist.tile([128, ET, D_LOC], f16)
            wo_sb = persist.tile([128, MT, SIZE], f16)
            bq_sb = persist.tile([128, MT], f32)
            vT_all = persist.tile([128, ET, S], f16)

            qh_sb = persist.tile([128, MT, S], f16)   # [dim within pair, pair, s]
            kh_sb = persist.tile([128, MT, S], f16)
            # V head tiles + ones column: [s%128, s//128, head, d(64)+1]
            vh65 = persist.tile([128, ST, H_LOC, D + 1], f16)
            ctx_sb = persist.tile([128, MT, S], f16)  # normalized ctxT, head pairs
            ones_f32 = persist.tile([128, 1], f32)
            nc.vector.memset(ones_f32[:], 1.0)
            ones_f16 = persist.tile([128, 1], f16)
            nc.vector.tensor_copy(ones_f16[:], ones_f32[:])
            # ones column of vh65 (stride-0 broadcast of the ones col)
            nc.vector.tensor_copy(
                vh65[:, :, :, D:D + 1].squeeze(-1),
                ones_f16.broadcast_to([128, ST, H_LOC]))
            # warm the exp table set while K projection runs
            warm = persist.tile([1, 1], f16)
            nc.scalar.activation(warm[:], ones_f32[0:1, :], Exp)
            junk = persist.tile([128, 512], f16)
            nc.vector.memset(junk[:], 0.5)

            # ---------- PE clock warm-up on junk data during the DMA lead-in
            with tc.tile_pool(name="psW", bufs=1, space="PSUM") as psW:
                wjunk = psW.tile([128, 512], f32, name="wjunk")
                for i in range(10):
                    nc.tensor.matmul(wjunk[:], junk[:, 0:128], junk[:],
                                     start=(i == 0), stop=(i == 9))

            # ---------- K projection (full), 8 PSUM banks ----------
            with (
                tc.tile_pool(name="xinK", bufs=4) as xinK,
                tc.tile_pool(name="psK", bufs=1, space="PSUM") as psK,
            ):
                pss = [psK.tile([128, 512], f32, tag=f"pa{i}", name=f"pa{i}")
                       for i in range(8)]
                for et in range(ET):
                    xrow = xinK.tile([128, S], f16, tag="xrow")
                    # alternate DMA-issue queues so the kT stream is not
                    # serialized behind itself
                    k_eng = nc.sync if et % 2 == 0 else nc.scalar
                    k_eng.dma_start(xrow[:, 0:1024], kTt[:, et, 0:1024])
                    k_eng.dma_start(xrow[:, 1024:2048], kTt[:, et, 1024:2048])
                    for mt in range(MT):
                        for nt in range(QC):
                            nc.tensor.matmul(
                                pss[nt * MT + mt][:],
                                wk_sb[:, et, mt * 128:(mt + 1) * 128],
                                xrow[:, nt * 512:(nt + 1) * 512],
                                start=(et == 0), stop=(et == ET - 1))
                for nt in range(QC):
                    for mt in range(MT):
                        nc.vector.tensor_scalar_add(
                            kh_sb[:, mt, nt * 512:(nt + 1) * 512],
                            pss[nt * MT + mt][:], bk_sb[:, mt:mt + 1])

            # remaining preloads on the second (Activation) DMA queue, after
            # the odd-et kT rows it carries for the K projection
            nc.scalar.dma_start(wq_sb[:], WqT_d.rearrange("(et p) m -> p et m", p=128))
            nc.scalar.dma_start(bq_sb[:], bq_d.rearrange("(mt p) -> p mt", p=128))
            nc.scalar.dma_start(wv_sb[:], WvT_d.rearrange("(et p) m -> p et m", p=128))
            nc.scalar.dma_start(wo_sb[:], WoT_d.rearrange("(hp p) o -> p hp o", p=128))
            for et in range(ET):
                nc.scalar.dma_start(vT_all[:, et, :], vTt[:, et, :])

            # ---------- pipelined attention ----------
            with (
                tc.tile_pool(name="psS", bufs=3, space="PSUM") as psS,
                tc.tile_pool(name="psC", bufs=1, space="PSUM") as psC,
                tc.tile_pool(name="qin", bufs=8) as qin,
                tc.tile_pool(name="esb", bufs=LAG + 2) as esb,
                tc.tile_pool(name="smalls", bufs=2) as smalls,
                tc.tile_pool(name="osb", bufs=3) as osb,
                tc.tile_pool(name="dscr", bufs=2, space="DRAM") as dscr,
            ):
                def qproj_chunk(qc):
                    # one qrow load serves both mt blocks; mt-sequential so
                    # only one scores-pool slot is held at a time
                    qrows = []
                    for et in range(ET):
                        qrow = qin.tile([128, 512], f16, tag="qrow",
                                        name="qrow")
                        nc.sync.dma_start(
                            qrow[:], qTt[:, et, qc * 512:(qc + 1) * 512])
                        qrows.append(qrow)
                    for mt in range(MT):
                        psq = psS.tile([128, 512], f32, tag="sc", name="q")
                        for et in range(ET):
                            nc.tensor.matmul(
                                psq[:],
                                wq_sb[:, et, mt * 128:(mt + 1) * 128],
                                qrows[et][:],
                                start=(et == 0), stop=(et == ET - 1))
                        nc.vector.tensor_scalar_add(
                            qh_sb[:, mt, qc * 512:(qc + 1) * 512],
                            psq[:], bq_sb[:, mt:mt + 1])

                def vproj_st(st):
                    psv = psS.tile([128, D_LOC], f32, tag="sc", name="pv")
                    for et in range(ET):
                        nc.tensor.matmul(
                            psv[:],
                            vT_all[:, et, st * 128:(st + 1) * 128],
                            wv_sb[:, et, :],
                            start=(et == 0), stop=(et == ET - 1))
                    nc.vector.tensor_copy(
                        vh65[:, st, :, 0:D],
                        psv.rearrange("p (h d) -> p h d", h=H_LOC))

                def outproj_chunk(st, ot):
                    pso = psS.tile([128, 512], f32, tag="sc", name=f"po{ot}")
                    for hp in range(MT):
                        nc.tensor.matmul(
                            pso[:],
                            ctx_sb[:, hp, st * 128:(st + 1) * 128],
                            wo_sb[:, hp, ot * 512:(ot + 1) * 512],
                            start=(hp == 0), stop=(hp == MT - 1))
                    o_sb = osb.tile([128, 512], f32, tag="o", name="o")
                    nc.vector.tensor_copy(o_sb[:], pso[:])
                    nc.sync.dma_start(
                        out_d[st * 128:(st + 1) * 128,
                              ot * 512:(ot + 1) * 512], o_sb[:])

                def sweep(qc, pr, extras):
                    # normalize-chain DMAs ride the gpsimd SWDGE queue (off
                    # the in-order SP queue), except the last q-chunk whose
                    # chain is latency-exposed at the kernel tail - SP is
                    # idle there and lower-latency
                    dma_eng = nc.sync if qc == QC - 1 else nc.gpsimd
                    q0 = qc * 512
                    cpair = [psC.tile([D + 1, 512], f32, tag=f"c{hi}",
                                      name=f"c{hi}") for hi in range(2)]
                    pend = []

                    def emit_ctx(kt, e_sb):
                        for hi in range(2):
                            nc.tensor.matmul(
                                cpair[hi][:],
                                vh65[:, kt, 2 * pr + hi, :],
                                e_sb[:, hi * 512:(hi + 1) * 512],
                                start=(kt == 0), stop=(kt == KT - 1))

                    if qc == 0 and pr == 0:
                        dve_kts = ()          # V-proj sweep: PE-bound, Act idle
                    elif extras:
                        dve_kts = DVE_KTS_HEAVY
                    else:
                        dve_kts = DVE_KTS_LIGHT
                    for kt in range(KT):
                        for fn in extras.get(kt, ()):
                            fn()
                        scs = psS.tile([128, 1024], f32, tag="sc", name="sc")
                        for hi in range(2):
                            po = hi * D
                            nc.tensor.matmul(
                                scs[:, hi * 512:(hi + 1) * 512],
                                kh_sb[po:po + D, pr, kt * 128:(kt + 1) * 128],
                                qh_sb[po:po + D, pr, q0:q0 + 512],
                                start=True, stop=True)
                        if kt in dve_kts:
                            # Schraudolph exp on DVE: uint16(A*x+B) bitcast
                            # to fp16 ~= e^x; negatives saturate to 0
                            e_u = esb.tile([128, 1024], u16, tag="e", name="eu")
                            nc.vector.tensor_scalar(
                                out=e_u[:], in0=scs[:],
                                scalar1=SCH_A, scalar2=SCH_B,
                                op0=Alu.mult, op1=Alu.add)
                            e_sb = e_u[:].bitcast(f16)
                        else:
                            e_t = esb.tile([128, 1024], f16, tag="e", name="e")
                            nc.scalar.activation(e_t[:], scs[:], Exp)
                            e_sb = e_t[:]
                        pend.append((kt, e_sb))
                        if len(pend) > LAG:
                            emit_ctx(*pend.pop(0))
                    for item in pend:
                        emit_ctx(*item)

                    # evacuate ctx+denominator, reciprocal via 64-partition
                    # DRAM spread, broadcast, normalize
                    cus = [smalls.tile([D + 1, 512], f32, tag=f"cu{hi}",
                                       name=f"cu{hi}") for hi in range(2)]
                    for hi in range(2):
                        nc.vector.tensor_copy(cus[hi][:], cpair[hi][:])
                    scr = dscr.tile([1024], f32, tag="scr", name="scr")
                    for hi in range(2):
                        dma_eng.dma_start(
                            scr[hi * 512:(hi + 1) * 512].rearrange(
                                "(a b) -> a b", a=1),
                            cus[hi][D:D + 1, :])
                    spread = smalls.tile([64, 16], f32, tag="spread",
                                         name="spread")
                    dma_eng.dma_start(
                        spread[:], scr[:].rearrange("(p j) -> p j", p=64))
                    spread_r = smalls.tile([64, 16], f32, tag="spreadr",
                                           name="spreadr")
                    nc.vector.reciprocal(spread_r[:], spread[:])
                    scr2 = dscr.tile([1024], f32, tag="scr2", name="scr2")
                    dma_eng.dma_start(
                        scr2[:].rearrange("(p j) -> p j", p=64), spread_r[:])
                    brec = smalls.tile([64, 1024], f32, tag="brec", name="brec")
                    dma_eng.dma_start(
                        brec[:],
                        bass.AP(tensor=scr2.tensor, offset=scr2.offset,
                                ap=[[0, 64]] + list(scr2[:].ap)))
                    # h0 -> ctx_sb rows 0-63 directly; h1 -> staging then
                    # SBUF->SBUF DMA shift to rows 64-127
                    nc.gpsimd.tensor_mul(
                        ctx_sb[0:D, pr, q0:q0 + 512],
                        cus[0][0:D, :], brec[:, 0:512])
                    stg = smalls.tile([D, 512], f16, tag="stg", name="stg")
                    nc.gpsimd.tensor_mul(
                        stg[:], cus[1][0:D, :], brec[:, 512:1024])
                    dma_eng.dma_start(
                        ctx_sb[D:2 * D, pr, q0:q0 + 512], stg[:])

                qproj_chunk(0)
                for qc in range(QC):
                    # first sweep carries the interleaved V projection; each
                    # pr=1 sweep carries the PREVIOUS q-chunk's output
                    # projection (one full sweep of slack behind its
                    # normalize chain)
                    extras0 = ({kt: (lambda st=kt: vproj_st(st),)
                                for kt in range(KT)} if qc == 0 else {})
                    sweep(qc, 0, extras0)
                    if qc + 1 < QC:
                        qproj_chunk(qc + 1)
                    if qc == 0:
                        extras1 = {}
                    else:
                        extras1 = {2 * i + 1:
                                   (lambda st=(qc - 1) * 4 + i // 2,
                                    ot=i % 2: outproj_chunk(st, ot),)
                                   for i in range(8)}
                    sweep(qc, 1, extras1)
                # tail: last q-chunk's output projection
                for st4 in range(4):
                    for ot in range(2):
                        outproj_chunk(12 + st4, ot)

    nc.compile()
    _NC = nc
    return nc


def prepare_in_maps(inputs):
    q, k, v = inputs["q"], inputs["k"], inputs["v"]
    Wq, bq = inputs["Wq"], inputs["bq"]
    Wk, bk = inputs["Wk"], inputs["bk"]
    Wv = inputs["Wv"]
    Wo = inputs["Wo"]
    sc = np.float32(1.0 / np.sqrt(D))

    f32, f16 = np.float32, np.float16
    qT = [q[b].T.astype(f16) for b in range(B)]
    kT = [k[b].T.astype(f16) for b in range(B)]
    vT = [v[b].T.astype(f16) for b in range(B)]
    WqTs = (Wq.T * sc).astype(f16)   # scale folded into Wq
    WkT = Wk.T.astype(f16)
    WvT = Wv.T.astype(f16)
    WoT = Wo.T.astype(f16)           # [c, o]
    bqs = (bq * sc).astype(f32)

    in_maps = []
    for core in range(NCORES):
        b, hg = divmod(core, HGROUPS)
        sl = slice(hg * D_LOC, (hg + 1) * D_LOC)
        in_maps.append({
            "qT": qT[b], "kT": kT[b], "vT": vT[b],
            "WqT": np.ascontiguousarray(WqTs[:, sl]),
            "WkT": np.ascontiguousarray(WkT[:, sl]),
            "WvT": np.ascontiguousarray(WvT[:, sl]),
            "WoT": np.ascontiguousarray(WoT[sl, :]),
            "bq": np.ascontiguousarray(bqs[sl]),
            "bk": np.ascontiguousarray(bk[sl].astype(f32)),
        })
    return in_maps


def gather(results, inputs):
    # host epilogue: sum the 4 tensor-parallel partials per batch and add the
    # constant row bv @ Wo.T + bo (the value bias commutes through softmax)
    const = (inputs["bv"].astype(np.float64) @ inputs["Wo"].astype(np.float64).T
             + inputs["bo"].astype(np.float64)).astype(np.float32)
    full = np.empty((B, S, SIZE), np.float32)
    for b in range(B):
        acc = results[b * HGROUPS]["out"].astype(np.float32).copy()
        for hg in range(1, HGROUPS):
            acc += results[b * HGROUPS + hg]["out"]
        full[b] = acc + const[None, :]
    return full


def kernel(**inputs):
    nc = build()
    in_maps = prepare_in_maps(inputs)
    res = run_bass_kernel_spmd(nc, in_maps, core_ids=list(range(NCORES)), trace=False)
    return gather(res.results, inputs)


# revision 11
# speedup vs baseline: 1.0366x; 1.0366x over previous
"""Fused multi-head attention forward (B=2, S=2048, SIZE=1024, H=16) on 8
Trainium2 NeuronCores.

Sharding: 2-way data parallel over batch x 4-way tensor parallel over heads
(Megatron style). Each core computes 4 heads of one batch element end-to-end
(QKV projections for its 256-dim slice, attention, and a partial output
projection); the host sums the 4 partials per batch and adds the output
bias. The value-projection bias drops out of attention algebraically
(softmax rows sum to 1), so the host folds `bv @ Wo.T` into that same
constant row.

On-device layouts avoid all transposes (fp16 operands, fp32 PSUM):
  qhT/khT [dim, s]  <- host-transposed qT/kT as matmul rhs, WqT/WkT as lhsT
  vh65    [s%128, st, head, 65] <- V head tiles with a trailing ONES column:
                       the ctx matmul's output row 64 then accumulates the
                       softmax denominator for free
  scoresT [k, q]    <- khT as lhsT, qhT as rhs; head pairs stream through
                       the PE concurrently on disjoint row groups
  ctxT    [65, q]   <- vh65 as lhsT, exp(scoresT) as rhs
  out     [s, o]    <- ctxT as lhsT, WoT as rhs (c = 128, heads paired)

Schedule: one software pipeline paced by the Activation engine (exp of
S*S*H_loc elements at 1/lane/cycle is the largest single-engine load).
K projection runs up front while vT/weight DMAs stream on the second
DMA-issue queue (Activation's); attention sweeps over (q-chunk, head-pair)
follow. Within a sweep the ctx matmuls are issued TWO kt slots behind the
exp that feeds them so their PE issue is never gated on an Act semaphore;
the scores PSUM pool is 3 deep for the same reason. The V projection is
interleaved into the first sweep's slots; each q-chunk's output projection
is spread in 2-matmul chunks across the next sweep's slots. Softmax
denominators take one DRAM spread/broadcast roundtrip (64-partition
reciprocal); odd heads' normalized ctx takes an SBUF->SBUF DMA to shift to
partitions 64-127 for the paired output projection.
"""

import numpy as np

import concourse.bass as bass
import concourse.tile as tile
from concourse import bacc, mybir
from concourse.bass_utils import run_bass_kernel_spmd

B, S, SIZE, H, D = 2, 2048, 1024, 16, 64
NCORES = 8
HGROUPS = 4                # tensor-parallel head groups
H_LOC = H // HGROUPS       # 4 heads per core
D_LOC = H_LOC * D          # 256 projection dims per core
MT = D_LOC // 128          # 2 head-pairs per core
ET = SIZE // 128           # 8 contraction tiles for projections
ST = S // 128              # 16 sequence tiles of 128
QC = S // 512              # 4 q-chunks of 512
KT = S // 128              # 16 key tiles
LAG = 2                    # ctx matmuls trail exp by this many kt slots
DVE_KTS_LIGHT = (3, 5, 8, 11, 14)  # DVE exp slots in extras-free sweeps
DVE_KTS_HEAVY = (5, 11)            # DVE exp slots in out-proj-carrying sweeps
SCH_A = float(2 ** 10 / np.log(2.0))   # fp16 Schraudolph exp: uint16(A*x+B)
SCH_B = 15315.25                       # bitcast to fp16 ~= e^x (max rel 3.0%)

_NC = None


def build():
    global _NC
    if _NC is not None:
        return _NC
    f32, f16 = mybir.dt.float32, mybir.dt.float16
    u16 = mybir.dt.uint16
    Exp = mybir.ActivationFunctionType.Exp
    Alu = mybir.AluOpType

    nc = bacc.Bacc("TRN2", target_bir_lowering=False, debug=False)
    qT_d = nc.dram_tensor("qT", [SIZE, S], f16, kind="ExternalInput").ap()
    kT_d = nc.dram_tensor("kT", [SIZE, S], f16, kind="ExternalInput").ap()
    vT_d = nc.dram_tensor("vT", [SIZE, S], f16, kind="ExternalInput").ap()
    WqT_d = nc.dram_tensor("WqT", [SIZE, D_LOC], f16, kind="ExternalInput").ap()
    WkT_d = nc.dram_tensor("WkT", [SIZE, D_LOC], f16, kind="ExternalInput").ap()
    WvT_d = nc.dram_tensor("WvT", [SIZE, D_LOC], f16, kind="ExternalInput").ap()
    WoT_d = nc.dram_tensor("WoT", [D_LOC, SIZE], f16, kind="ExternalInput").ap()
    bq_d = nc.dram_tensor("bq", [D_LOC], f32, kind="ExternalInput").ap()
    bk_d = nc.dram_tensor("bk", [D_LOC], f32, kind="ExternalInput").ap()
    out_d = nc.dram_tensor("out", [S, SIZE], f16, kind="ExternalOutput").ap()

    qTt = qT_d.rearrange("(et p) s -> p et s", p=128)
    kTt = kT_d.rearrange("(et p) s -> p et s", p=128)
    vTt = vT_d.rearrange("(et p) s -> p et s", p=128)

    with tile.TileContext(nc) as tc:
        with tc.tile_pool(name="persist", bufs=1) as persist:
            wk_sb = persist.tile([128, ET, D_LOC], f16)
            bk_sb = persist.tile([128, MT], f32)
            WkTt = WkT_d.rearrange("(et p) m -> p et m", p=128)
            for et in range(ET):   # per-et chunks: K matmuls start early
                nc.sync.dma_start(wk_sb[:, et, :], WkTt[:, et, :])
            nc.scalar.dma_start(bk_sb[:], bk_d.rearrange("(mt p) -> p mt", p=128))
            wq_sb = persist.tile([128, ET, D_LOC], f16)
            wv_sb = persist.tile([128, ET, D_LOC], f16)
            wo_sb = persist.tile([128, MT, SIZE], f16)
            bq_sb = persist.tile([128, MT], f32)
            vT_all = persist.tile([128, ET, S], f16)

            qh_sb = persist.tile([128, MT, S], f16)   # [dim within pair, pair, s]
            kh_sb = persist.tile([128, MT, S], f16)
            # V head tiles + ones column: [s%128, s//128, head, d(64)+1]
            vh65 = persist.tile([128, ST, H_LOC, D + 1], f16)
            ctx_sb = persist.tile([128, MT, S], f16)  # normalized ctxT, head pairs
            ones_f32 = persist.tile([128, 1], f32)
            nc.vector.memset(ones_f32[:], 1.0)
            ones_f16 = persist.tile([128, 1], f16)
            nc.vector.tensor_copy(ones_f16[:], ones_f32[:])
            # ones column of vh65 (stride-0 broadcast of the ones col)
            nc.vector.tensor_copy(
                vh65[:, :, :, D:D + 1].squeeze(-1),
                ones_f16.broadcast_to([128, ST, H_LOC]))
            # warm the exp table set while K projection runs
            warm = persist.tile([1, 1], f16)
            nc.scalar.activation(warm[:], ones_f32[0:1, :], Exp)
            junk = persist.tile([128, 512], f16)
            nc.vector.memset(junk[:], 0.5)

            # ---------- PE clock warm-up on junk data during the DMA lead-in
            with tc.tile_pool(name="psW", bufs=1, space="PSUM") as psW:
                wjunk = psW.tile([128, 512], f32, name="wjunk")
                for i in range(10):
                    nc.tensor.matmul(wjunk[:], junk[:, 0:128], junk[:],
                                     start=(i == 0), stop=(i == 9))

            # ---------- K projection (full), 8 PSUM banks ----------
            with (
                tc.tile_pool(name="xinK", bufs=4) as xinK,
                tc.tile_pool(name="psK", bufs=1, space="PSUM") as psK,
            ):
                pss = [psK.tile([128, 512], f32, tag=f"pa{i}", name=f"pa{i}")
                       for i in range(8)]
                for et in range(ET):
                    xrow = xinK.tile([128, S], f16, tag="xrow")
                    # alternate DMA-issue queues so the kT stream is not
                    # serialized behind itself
                    k_eng = nc.sync if et % 2 == 0 else nc.scalar
                    k_eng.dma_start(xrow[:, 0:1024], kTt[:, et, 0:1024])
                    k_eng.dma_start(xrow[:, 1024:2048], kTt[:, et, 1024:2048])
                    for mt in range(MT):
                        for nt in range(QC):
                            nc.tensor.matmul(
                                pss[nt * MT + mt][:],
                                wk_sb[:, et, mt * 128:(mt + 1) * 128],
                                xrow[:, nt * 512:(nt + 1) * 512],
                                start=(et == 0), stop=(et == ET - 1))
                for nt in range(QC):
                    for mt in range(MT):
                        nc.vector.tensor_scalar_add(
                            kh_sb[:, mt, nt * 512:(nt + 1) * 512],
                            pss[nt * MT + mt][:], bk_sb[:, mt:mt + 1])

            # remaining preloads: odd vT rows + wv early on the second
            # (Activation) DMA queue right after the odd-et kT rows; even vT
            # rows go on SP after the first q-chunk's rows (emitted below)
            nc.scalar.dma_start(wv_sb[:], WvT_d.rearrange("(et p) m -> p et m", p=128))
            for et in range(1, ET, 2):
                nc.scalar.dma_start(vT_all[:, et, :], vTt[:, et, :])
            nc.scalar.dma_start(wq_sb[:], WqT_d.rearrange("(et p) m -> p et m", p=128))
            nc.scalar.dma_start(bq_sb[:], bq_d.rearrange("(mt p) -> p mt", p=128))
            nc.scalar.dma_start(wo_sb[:], WoT_d.rearrange("(hp p) o -> p hp o", p=128))

            # ---------- pipelined attention ----------
            with (
                tc.tile_pool(name="psS", bufs=3, space="PSUM") as psS,
                tc.tile_pool(name="psC", bufs=1, space="PSUM") as psC,
                tc.tile_pool(name="qin", bufs=8) as qin,
                tc.tile_pool(name="esb", bufs=LAG + 2) as esb,
                tc.tile_pool(name="smalls", bufs=2) as smalls,
                tc.tile_pool(name="osb", bufs=3) as osb,
                tc.tile_pool(name="dscr", bufs=2, space="DRAM") as dscr,
            ):
                def qproj_chunk(qc):
                    # one qrow load serves both mt blocks; mt-sequential so
                    # only one scores-pool slot is held at a time
                    qrows = []
                    for et in range(ET):
                        qrow = qin.tile([128, 512], f16, tag="qrow",
                                        name="qrow")
                        nc.sync.dma_start(
                            qrow[:], qTt[:, et, qc * 512:(qc + 1) * 512])
                        qrows.append(qrow)
                    for mt in range(MT):
                        psq = psS.tile([128, 512], f32, tag="sc", name="q")
                        for et in range(ET):
                            nc.tensor.matmul(
                                psq[:],
                                wq_sb[:, et, mt * 128:(mt + 1) * 128],
                                qrows[et][:],
                                start=(et == 0), stop=(et == ET - 1))
                        nc.vector.tensor_scalar_add(
                            qh_sb[:, mt, qc * 512:(qc + 1) * 512],
                            psq[:], bq_sb[:, mt:mt + 1])

                def vproj_st(st):
                    psv = psS.tile([128, D_LOC], f32, tag="sc", name="pv")
                    for et in range(ET):
                        nc.tensor.matmul(
                            psv[:],
                            vT_all[:, et, st * 128:(st + 1) * 128],
                            wv_sb[:, et, :],
                            start=(et == 0), stop=(et == ET - 1))
                    nc.vector.tensor_copy(
                        vh65[:, st, :, 0:D],
                        psv.rearrange("p (h d) -> p h d", h=H_LOC))

                def outproj_chunk(st, ot):
                    pso = psS.tile([128, 512], f32, tag="sc", name=f"po{ot}")
                    for hp in range(MT):
                        nc.tensor.matmul(
                            pso[:],
                            ctx_sb[:, hp, st * 128:(st + 1) * 128],
                            wo_sb[:, hp, ot * 512:(ot + 1) * 512],
                            start=(hp == 0), stop=(hp == MT - 1))
                    o_sb = osb.tile([128, 512], f16, tag="o", name="o")
                    nc.vector.tensor_copy(o_sb[:], pso[:])
                    nc.sync.dma_start(
                        out_d[st * 128:(st + 1) * 128,
                              ot * 512:(ot + 1) * 512], o_sb[:])

                def sweep(qc, pr, extras):
                    # normalize-chain DMAs ride the gpsimd SWDGE queue (off
                    # the in-order SP queue), except the last q-chunk whose
                    # chain is latency-exposed at the kernel tail - SP is
                    # idle there and lower-latency
                    dma_eng = nc.sync if qc == QC - 1 else nc.gpsimd
                    q0 = qc * 512
                    cpair = [psC.tile([D + 1, 512], f32, tag=f"c{hi}",
                                      name=f"c{hi}") for hi in range(2)]
                    pend = []

                    def emit_ctx(kt, e_sb):
                        for hi in range(2):
                            nc.tensor.matmul(
                                cpair[hi][:],
                                vh65[:, kt, 2 * pr + hi, :],
                                e_sb[:, hi * 512:(hi + 1) * 512],
                                start=(kt == 0), stop=(kt == KT - 1))

                    if qc == 0 and pr == 0:
                        dve_kts = ()          # V-proj sweep: PE-bound, Act idle
                    elif extras:
                        dve_kts = DVE_KTS_HEAVY
                    else:
                        dve_kts = DVE_KTS_LIGHT
                    for kt in range(KT):
                        for fn in extras.get(kt, ()):
                            fn()
                        scs = psS.tile([128, 1024], f32, tag="sc", name="sc")
                        for hi in range(2):
                            po = hi * D
                            nc.tensor.matmul(
                                scs[:, hi * 512:(hi + 1) * 512],
                                kh_sb[po:po + D, pr, kt * 128:(kt + 1) * 128],
                                qh_sb[po:po + D, pr, q0:q0 + 512],
                                start=True, stop=True)
                        if kt in dve_kts:
                            # Schraudolph exp on DVE: uint16(A*x+B) bitcast
                            # to fp16 ~= e^x; negatives saturate to 0
                            e_u = esb.tile([128, 1024], u16, tag="e", name="eu")
                            nc.vector.tensor_scalar(
                                out=e_u[:], in0=scs[:],
                                scalar1=SCH_A, scalar2=SCH_B,
                                op0=Alu.mult, op1=Alu.add)
                            e_sb = e_u[:].bitcast(f16)
                        else:
                            e_t = esb.tile([128, 1024], f16, tag="e", name="e")
                            nc.scalar.activation(e_t[:], scs[:], Exp)
                            e_sb = e_t[:]
                        pend.append((kt, e_sb))
                        if len(pend) > LAG:
                            emit_ctx(*pend.pop(0))
                    for item in pend:
                        emit_ctx(*item)

                    # evacuate ctx+denominator, reciprocal via 64-partition
                    # DRAM spread, broadcast, normalize
                    cus = [smalls.tile([D + 1, 512], f32, tag=f"cu{hi}",
                                       name=f"cu{hi}") for hi in range(2)]
                    for hi in range(2):
                        nc.vector.tensor_copy(cus[hi][:], cpair[hi][:])
                    scr = dscr.tile([1024], f32, tag="scr", name="scr")
                    for hi in range(2):
                        dma_eng.dma_start(
                            scr[hi * 512:(hi + 1) * 512].rearrange(
                                "(a b) -> a b", a=1),
                            cus[hi][D:D + 1, :])
                    spread = smalls.tile([64, 16], f32, tag="spread",
                                         name="spread")
                    dma_eng.dma_start(
                        spread[:], scr[:].rearrange("(p j) -> p j", p=64))
                    spread_r = smalls.tile([64, 16], f32, tag="spreadr",
                                           name="spreadr")
                    nc.vector.reciprocal(spread_r[:], spread[:])
                    scr2 = dscr.tile([1024], f32, tag="scr2", name="scr2")
                    dma_eng.dma_start(
                        scr2[:].rearrange("(p j) -> p j", p=64), spread_r[:])
                    brec = smalls.tile([64, 1024], f32, tag="brec", name="brec")
                    dma_eng.dma_start(
                        brec[:],
                        bass.AP(tensor=scr2.tensor, offset=scr2.offset,
                                ap=[[0, 64]] + list(scr2[:].ap)))
                    # h0 -> ctx_sb rows 0-63 directly; h1 -> staging then
                    # SBUF->SBUF DMA shift to rows 64-127
                    nc.gpsimd.tensor_mul(
                        ctx_sb[0:D, pr, q0:q0 + 512],
                        cus[0][0:D, :], brec[:, 0:512])
                    stg = smalls.tile([D, 512], f16, tag="stg", name="stg")
                    nc.gpsimd.tensor_mul(
                        stg[:], cus[1][0:D, :], brec[:, 512:1024])
                    dma_eng.dma_start(
                        ctx_sb[D:2 * D, pr, q0:q0 + 512], stg[:])

                qproj_chunk(0)
                for et in range(0, ET, 2):
                    nc.sync.dma_start(vT_all[:, et, :], vTt[:, et, :])
                for qc in range(QC):
                    # first sweep carries the interleaved V projection; each
                    # pr=1 sweep carries the PREVIOUS q-chunk's output
                    # projection (one full sweep of slack behind its
                    # normalize chain)
                    extras0 = ({kt: (lambda st=kt: vproj_st(st),)
                                for kt in range(KT)} if qc == 0 else {})
                    sweep(qc, 0, extras0)
                    if qc + 1 < QC:
                        qproj_chunk(qc + 1)
                    if qc == 0:
                        extras1 = {}
                    else:
                        extras1 = {2 * i + 1:
                                   (lambda st=(qc - 1) * 4 + i // 2,
                                    ot=i % 2: outproj_chunk(st, ot),)
                                   for i in range(8)}
                    sweep(qc, 1, extras1)
                # tail: last q-chunk's output projection
                for st4 in range(4):
                    for ot in range(2):
                        outproj_chunk(12 + st4, ot)

    nc.compile()
    _NC = nc
    return nc


def prepare_in_maps(inputs):
    q, k, v = inputs["q"], inputs["k"], inputs["v"]
    Wq, bq = inputs["Wq"], inputs["bq"]
    Wk, bk = inputs["Wk"], inputs["bk"]
    Wv = inputs["Wv"]
    Wo = inputs["Wo"]
    sc = np.float32(1.0 / np.sqrt(D))

    f32, f16 = np.float32, np.float16
    qT = [q[b].T.astype(f16) for b in range(B)]
    kT = [k[b].T.astype(f16) for b in range(B)]
    vT = [v[b].T.astype(f16) for b in range(B)]
    WqTs = (Wq.T * sc).astype(f16)   # scale folded into Wq
    WkT = Wk.T.astype(f16)
    WvT = Wv.T.astype(f16)
    WoT = Wo.T.astype(f16)           # [c, o]
    bqs = (bq * sc).astype(f32)

    in_maps = []
    for core in range(NCORES):
        b, hg = divmod(core, HGROUPS)
        sl = slice(hg * D_LOC, (hg + 1) * D_LOC)
        in_maps.append({
            "qT": qT[b], "kT": kT[b], "vT": vT[b],
            "WqT": np.ascontiguousarray(WqTs[:, sl]),
            "WkT": np.ascontiguousarray(WkT[:, sl]),
            "WvT": np.ascontiguousarray(WvT[:, sl]),
            "WoT": np.ascontiguousarray(WoT[sl, :]),
            "bq": np.ascontiguousarray(bqs[sl]),
            "bk": np.ascontiguousarray(bk[sl].astype(f32)),
        })
    return in_maps


def gather(results, inputs):
    # host epilogue: sum the 4 tensor-parallel partials per batch and add the
    # constant row bv @ Wo.T + bo (the value bias commutes through softmax)
    const = (inputs["bv"].astype(np.float64) @ inputs["Wo"].astype(np.float64).T
             + inputs["bo"].astype(np.float64)).astype(np.float32)
    full = np.empty((B, S, SIZE), np.float32)
    for b in range(B):
        acc = results[b * HGROUPS]["out"].astype(np.float32).copy()
        for hg in range(1, HGROUPS):
            acc += results[b * HGROUPS + hg]["out"]
        full[b] = acc + const[None, :]
    return full


def kernel(**inputs):
    nc = build()
    in_maps = prepare_in_maps(inputs)
    res = run_bass_kernel_spmd(nc, in_maps, core_ids=list(range(NCORES)), trace=False)
    return gather(res.results, inputs)
